# revision 2
# baseline (speedup 1.0000x reference)
"""Trainium2 Bass kernel for nn_Algebraic_65970697666729 (segment_reduce).

Computes, for x of shape (131072, 16) fp32:
    out = concat([x, all C(16,2)=120 pairwise products, all C(16,3)=560
                  triple products], axis=1)  -> (131072, 696) fp32

Sharding: pure data parallel over rows; 8 cores x 16384 rows each.

v2 design (from ntff trace analysis of v1):
  * The run is jointly limited by DVE supply (2x packed mode: 0.5208
    ns/elem/partition -> 45.3 us for the 87040 products per partition)
    and the 16-engine DMA drain (~435 GB/s -> 52.4 us for 22.3 MB/core).
    v1 spent ~19 us above the DVE floor on per-instruction overhead (66
    muls across 2 row-blocks) plus a late DMA ramp and a 9 us tail.
  * v2 uses ONE 128-row block: 36 muls total, each 2x as large, cutting
    fixed per-instruction cost in half, and 12 output DMA sections sized
    so the drain queue is fed from ~2 us onward and never starves.
  * Products stored bf16 (rel-err ~3.5e-3 vs the 2e-2 gate); the 16
    passthrough x columns are stitched on host from the fp32 input.
  * Compute in transposed per-partition layout [cols, rows]: rows
    innermost with stride 1 for all three operands keeps the DVE in its
    2x packed mode (broadcast factor sits on the unchecked outer dim).
  * x is prefetched in two halves (cols 8:16 first) so the i>=8 pair
    muls start ~0.6 us before the full input has landed.
  * Every DMA'd section lives in its own tile, so section DMAs and
    later DVE writes never share a tile (no false WAR deps).

Column layout (within the 680 device columns):
  pairs (i,j) i<j at po[i]..po[i+1]-1: pa = pairs i=0 (cols 0:15),
  pb1 = pairs i=1..7 (cols 15:92), pb2 = pairs i=8..14 (cols 92:120).
  triples (i,j,k) at 120+to[i]..: triples with first index i are
  bcast(x_i) * (pairs suffix po[i+1]:120), split where the suffix
  crosses the pb1/pb2 boundary (i<=6).
"""

import numpy as np

N_CORES = 8
ROWS_TOTAL = 131072
ROWS = ROWS_TOTAL // N_CORES  # 16384
N = 16
NPAIRS = 120
NTRIPLES = 560
OUT_DEV = NPAIRS + NTRIPLES  # 680 product columns stored by the device
OUT_FULL = N + OUT_DEV  # 696
P = 128
R = ROWS // P  # 128 rows per partition

# pairs section split (pair-run first indices)
PA_I = [0]          # cols 0:15
PB1_I = list(range(1, 8))   # cols 15:92
PB2_I = list(range(8, 15))  # cols 92:120

# triple groups (ranges of first index i) -> one tile + one DMA each
TRI_GROUPS = [(0, 1), (1, 2), (2, 3), (3, 4), (4, 5), (5, 6), (6, 8), (8, 10), (10, 14)]

_CACHE = {}


def _pair_offsets():
    po = [0] * (N + 1)
    for i in range(1, N + 1):
        po[i] = po[i - 1] + (N - 1 - (i - 1))
    return po


def _triple_offsets():
    to = [0] * (N - 1)
    for i in range(1, N - 1):
        m = N - 1 - (i - 1)
        to[i] = to[i - 1] + m * (m - 1) // 2
    return to


def _build():
    import concourse.bacc as bacc
    import concourse.mybir as mybir
    from concourse import tile

    bf16 = mybir.dt.bfloat16
    nc = bacc.Bacc(
        "TRN2",
        target_bir_lowering=False,
        debug=False,
        enable_asserts=False,
        num_devices=N_CORES,
    )
    # Flat per-partition layouts, packed by the host:
    #   xin[p, f*R + r] = x[p*R + r, f]
    #   out[p, c*R + r] = product_col_c(row p*R + r)
    xin = nc.dram_tensor("x", [P, N * R], bf16, kind="ExternalInput")
    out = nc.dram_tensor("out", [P, OUT_DEV * R], bf16, kind="ExternalOutput")

    po = _pair_offsets()
    to = _triple_offsets()
    to_end = to + [NTRIPLES]

    PB1_C0 = po[PB1_I[0]]   # 15
    PB2_C0 = po[PB2_I[0]]   # 92
    PB1_W = PB2_C0 - PB1_C0  # 77
    PB2_W = NPAIRS - PB2_C0  # 28

    with tile.TileContext(nc) as tc:
        with tc.tile_pool(name="sp", bufs=1) as sp:
            xt = sp.tile([P, N, R], bf16, name="x")
            pa = sp.tile([P, po[1], R], bf16, name="pa")
            pb1 = sp.tile([P, PB1_W, R], bf16, name="pb1")
            pb2 = sp.tile([P, PB2_W, R], bf16, name="pb2")
            gts = [
                sp.tile([P, to_end[b] - to[a], R], bf16, name=f"g{a}")
                for a, b in TRI_GROUPS
            ]

            # Prefetch x in two halves on the scalar engine's DGE queue so
            # the i>=8 pair muls can start before the full input lands.
            for f0, f1 in ((8, 16), (0, 8)):
                src = xin.ap()[:, f0 * R : f1 * R].rearrange(
                    "p (f r) -> p f r", f=f1 - f0
                )
                nc.scalar.dma_start(out=xt[:, f0:f1, :], in_=src)

            def dma_cols(c0, ncols, src_ap):
                s = c0 * R
                dst = out.ap()[:, s : s + ncols * R].rearrange(
                    "p (c r) -> p c r", c=ncols
                )
                nc.sync.dma_start(out=dst, in_=src_ap)

            def pair_mul(i, dst_tile, dst_off):
                L = N - 1 - i
                nc.vector.tensor_mul(
                    out=dst_tile[:, dst_off : dst_off + L, :],
                    in0=xt[:, i + 1 : N, :],
                    in1=xt[:, i : i + 1, :].broadcast_to([P, L, R]),
                )

            # pairs i=8..14 (need only x cols 8:15) -> pb2, shipped first
            for i in PB2_I:
                pair_mul(i, pb2, po[i] - PB2_C0)
            dma_cols(PB2_C0, PB2_W, pb2[:])
            # pairs i=0 -> pa
            pair_mul(0, pa, 0)
            dma_cols(0, po[1], pa[:])
            # pairs i=1..7 -> pb1
            for i in PB1_I:
                pair_mul(i, pb1, po[i] - PB1_C0)
            dma_cols(PB1_C0, PB1_W, pb1[:])

            # triples: group g covers first indices [ia, ib); one tile+DMA
            for g, (ia, ib) in enumerate(TRI_GROUPS):
                gt = gts[g]
                base = to[ia]
                for i in range(ia, ib):
                    m = N - 1 - i
                    L = m * (m - 1) // 2
                    a = to[i] - base
                    x1 = xt[:, i : i + 1, :]
                    if po[i + 1] < PB2_C0:
                        # pairs suffix crosses the pb1/pb2 boundary
                        La = PB2_C0 - po[i + 1]
                        nc.vector.tensor_mul(
                            out=gt[:, a : a + La, :],
                            in0=pb1[:, po[i + 1] - PB1_C0 : PB1_W, :],
                            in1=x1.broadcast_to([P, La, R]),
                        )
                        nc.vector.tensor_mul(
                            out=gt[:, a + La : a + L, :],
                            in0=pb2[:, 0:PB2_W, :],
                            in1=x1.broadcast_to([P, PB2_W, R]),
                        )
                    else:
                        nc.vector.tensor_mul(
                            out=gt[:, a : a + L, :],
                            in0=pb2[:, po[i + 1] - PB2_C0 : PB2_W, :],
                            in1=x1.broadcast_to([P, L, R]),
                        )
                dma_cols(NPAIRS + to[ia], to_end[ib] - to[ia], gt[:])

    nc.compile()
    return nc


def _run(x, trace=False, **spmd_kwargs):
    import ml_dtypes
    from concourse.bass_utils import run_bass_kernel_spmd

    if "nc" not in _CACHE:
        _CACHE["nc"] = _build()
    nc = _CACHE["nc"]

    x = np.ascontiguousarray(np.asarray(x, dtype=np.float32))
    assert x.shape == (ROWS_TOTAL, N), x.shape
    xb = x.astype(ml_dtypes.bfloat16)
    # [cores, P, R, N] -> [cores, P, N, R] f-major flat
    x4 = xb.reshape(N_CORES, P, R, N).transpose(0, 1, 3, 2)
    in_maps = [
        {"x": np.ascontiguousarray(x4[i]).reshape(P, N * R)} for i in range(N_CORES)
    ]
    res = run_bass_kernel_spmd(
        nc, in_maps, core_ids=list(range(N_CORES)), trace=trace, **spmd_kwargs
    )
    full = np.empty((ROWS_TOTAL, OUT_FULL), dtype=np.float32)
    full[:, :N] = x
    prod = full[:, N:].reshape(N_CORES, P, R, OUT_DEV)
    for i, r in enumerate(res.results):
        dev = np.asarray(r["out"]).reshape(P, OUT_DEV, R)
        prod[i] = dev.transpose(0, 2, 1).astype(np.float32)
    return full, res


def kernel(x):
    return _run(x)[0]


# revision 3
# speedup vs baseline: 1.0382x; 1.0382x over previous
"""Trainium2 Bass kernel for nn_Algebraic_65970697666729 (segment_reduce).

Computes, for x of shape (131072, 16) fp32:
    out = concat([x, all C(16,2)=120 pairwise products, all C(16,3)=560
                  triple products], axis=1)  -> (131072, 696) fp32

Sharding: pure data parallel over rows; 8 cores x 16384 rows each.

v3 design (from ntff trace analysis of v1/v2):
  * Jointly limited by DVE supply (2x packed mode, 0.5208 ns/elem/
    partition -> 45.3 us floor for 87040 products/partition) and the
    16-engine DMA drain (~435 GB/s -> 51.2 us for 22.3 MB/core), after
    a fixed ~7 us framework preamble. The schedule must keep the drain
    queue non-empty from first section to last.
  * ONE 128-row block, 39 muls (v1 had 66 across 2 blocks; each mul
    carries ~150 ns fixed cost, largely pipelined away by the DVE's
    8-deep exec queue).
  * Output DMA sections capped at <= ~40 columns (~1.3 MB) so the
    drain never idles waiting for a big section to finish computing
    (v2's 105-col sections starved it). Every section lives in its own
    tile so section DMAs and later DVE writes never share a tile.
  * x is prefetched in two halves on two different DGE queues (scalar:
    cols 8:16, sync: cols 0:8) so the i>=8 pair muls start as soon as
    the first half lands.
  * Products stored bf16 (rel-err ~3.5e-3 vs the 2e-2 gate); the 16
    passthrough x columns are stitched on host from the fp32 input.
  * Compute in transposed per-partition layout [cols, rows]: rows
    innermost with stride 1 for all three operands keeps the DVE in
    its 2x packed mode (broadcast factor sits on the outer dim).

Column layout (680 device columns): pairs (i,j) i<j at po[i]..; pair
sections pa = i=0 (cols 0:15), pb1a = i=1..3 (15:54), pb1b = i=4..7
(54:92), pb2 = i=8..14 (92:120). triples with first index i are
bcast(x_i) * (pairs suffix po[i+1]:120) at 120+to[i].., computed in
parts split at the pair-section boundaries (54, 92).
"""

import numpy as np

N_CORES = 8
ROWS_TOTAL = 131072
ROWS = ROWS_TOTAL // N_CORES  # 16384
N = 16
NPAIRS = 120
NTRIPLES = 560
OUT_DEV = NPAIRS + NTRIPLES  # 680 product columns stored by the device
OUT_FULL = N + OUT_DEV  # 696
P = 128
R = ROWS // P  # 128 rows per partition

_CACHE = {}


def _pair_offsets():
    po = [0] * (N + 1)
    for i in range(1, N + 1):
        po[i] = po[i - 1] + (N - 1 - (i - 1))
    return po


def _triple_offsets():
    to = [0] * N
    for i in range(1, N):
        m = N - 1 - (i - 1)
        to[i] = to[i - 1] + m * (m - 1) // 2
    return to


# pair sections: (first-index range) -> columns [po[a], po[b])
PAIR_SECS = [(8, 15), (0, 1), (1, 4), (4, 8)]  # pb2, pa, pb1a, pb1b
# triple sections: list of (i, pair-col lo, pair-col hi) per tile; each
# tuple is one mul (triple parts of first-index i over that pair range).
# Boundaries at pair cols 54 and 92 (the pair tile splits).


def _triple_secs():
    po = _pair_offsets()
    bounds = [po[1], po[4], po[8], NPAIRS]  # 15, 54, 92, 120
    secs = []  # each: list of (i, lo, hi)
    for i in range(0, 7):  # crossing triples, one section per part
        lo = po[i + 1]
        cuts = [b for b in bounds if lo < b <= NPAIRS]
        parts = []
        for b in cuts:
            parts.append((i, lo, b))
            lo = b
        if i <= 3:
            for p_ in parts:
                secs.append([p_])  # big: own section each
        else:
            secs.append(parts)  # i=4..6: merge parts into one section
    secs.append([(7, po[8], NPAIRS)])
    secs.append([(8, po[9], NPAIRS), (9, po[10], NPAIRS)])
    secs.append([(i, po[i + 1], NPAIRS) for i in range(10, 14)])
    return secs


def _build():
    import concourse.bacc as bacc
    import concourse.mybir as mybir
    from concourse import tile

    bf16 = mybir.dt.bfloat16
    nc = bacc.Bacc(
        "TRN2",
        target_bir_lowering=False,
        debug=False,
        enable_asserts=False,
        num_devices=N_CORES,
    )
    # Flat per-partition layouts, packed by the host:
    #   xin[p, f*R + r] = x[p*R + r, f]
    #   out[p, c*R + r] = product_col_c(row p*R + r)
    xin = nc.dram_tensor("x", [P, N * R], bf16, kind="ExternalInput")
    out = nc.dram_tensor("out", [P, OUT_DEV * R], bf16, kind="ExternalOutput")

    po = _pair_offsets()
    to = _triple_offsets()
    tsecs = _triple_secs()

    with tile.TileContext(nc) as tc:
        with tc.tile_pool(name="sp", bufs=1) as sp:
            xt = sp.tile([P, N, R], bf16, name="x")
            psecs = []  # (a, b, tile) pair sections
            for a, b in PAIR_SECS:
                w = po[b] - po[a]
                psecs.append((a, b, sp.tile([P, w, R], bf16, name=f"p{a}")))
            tts = []
            for k, parts in enumerate(tsecs):
                w = sum(hi - lo for _, lo, hi in parts)
                tts.append(sp.tile([P, w, R], bf16, name=f"t{k}"))

            def pcol_ap(lo, hi):
                # AP over pair columns [lo, hi) from the owning section tile
                for a, b, t in psecs:
                    if po[a] <= lo and hi <= po[b]:
                        return t[:, lo - po[a] : hi - po[a], :]
                raise AssertionError((lo, hi))

            # x prefetch: cols 8:16 on the scalar queue, 0:8 on sync, so
            # the pb2 muls can start as soon as the first half lands.
            nc.scalar.dma_start(
                out=xt[:, 8:16, :],
                in_=xin.ap()[:, 8 * R : 16 * R].rearrange("p (f r) -> p f r", f=8),
            )
            nc.sync.dma_start(
                out=xt[:, 0:8, :],
                in_=xin.ap()[:, 0 : 8 * R].rearrange("p (f r) -> p f r", f=8),
            )

            def dma_cols(c0, ncols, src_ap):
                s = c0 * R
                dst = out.ap()[:, s : s + ncols * R].rearrange(
                    "p (c r) -> p c r", c=ncols
                )
                nc.sync.dma_start(out=dst, in_=src_ap)

            # pairs
            for a, b, t in psecs:
                for i in range(a, b):
                    L = N - 1 - i
                    nc.vector.tensor_mul(
                        out=t[:, po[i] - po[a] : po[i] - po[a] + L, :],
                        in0=xt[:, i + 1 : N, :],
                        in1=xt[:, i : i + 1, :].broadcast_to([P, L, R]),
                    )
                dma_cols(po[a], po[b] - po[a], t[:])

            # triples
            for parts, gt in zip(tsecs, tts):
                off = 0
                for i, lo, hi in parts:
                    w = hi - lo
                    nc.vector.tensor_mul(
                        out=gt[:, off : off + w, :],
                        in0=pcol_ap(lo, hi),
                        in1=xt[:, i : i + 1, :].broadcast_to([P, w, R]),
                    )
                    off += w
                i0, lo0, _ = parts[0]
                c0 = NPAIRS + to[i0] + (lo0 - po[i0 + 1])
                dma_cols(c0, off, gt[:])

    nc.compile()
    return nc


def _run(x, trace=False, **spmd_kwargs):
    import ml_dtypes
    from concourse.bass_utils import run_bass_kernel_spmd

    if "nc" not in _CACHE:
        _CACHE["nc"] = _build()
    nc = _CACHE["nc"]

    x = np.ascontiguousarray(np.asarray(x, dtype=np.float32))
    assert x.shape == (ROWS_TOTAL, N), x.shape
    xb = x.astype(ml_dtypes.bfloat16)
    x4 = xb.reshape(N_CORES, P, R, N).transpose(0, 1, 3, 2)
    in_maps = [
        {"x": np.ascontiguousarray(x4[i]).reshape(P, N * R)} for i in range(N_CORES)
    ]
    res = run_bass_kernel_spmd(
        nc, in_maps, core_ids=list(range(N_CORES)), trace=trace, **spmd_kwargs
    )
    full = np.empty((ROWS_TOTAL, OUT_FULL), dtype=np.float32)
    full[:, :N] = x
    prod = full[:, N:].reshape(N_CORES, P, R, OUT_DEV)
    for i, r in enumerate(res.results):
        dev = np.asarray(r["out"]).reshape(P, OUT_DEV, R)
        prod[i] = dev.transpose(0, 2, 1).astype(np.float32)
    return full, res


def kernel(x):
    return _run(x)[0]
